# revision 25
# baseline (speedup 1.0000x reference)
"""Causal self-attention (B=2, T=2048, C=768, H=12) on 8 Trainium2 cores.

Sharding: 24 (batch, head) pairs / 8 cores = 3 heads per core.
core c -> batch b = c // 4, heads [3g, 3g+3) with g = c % 4.

Per-core device program (identical SPMD program, different input data):
  qkT  = (Wqk_local^T @ x_b^T)          [384, T]   (q cols pre-scaled 1/8,
                                                    q bias added, k bias
                                                    dropped: softmax-invariant)
  V    = x_b @ Wv_local                  [T, 192]   (v bias folded on host)
  per head h:
    scoresT[k, q] = kT_h^T-block @ qT_h  (PE, K=64; diagonal blocks trimmed
                                          to the causally-needed q columns)
    expT = exp(scoresT)                  (ACT; diagonal blocks multiplied
                                          by precomputed 0/1 masks on DVE,
                                          trimmed cols zero-filled)
    y_augT[[d;1], q] += V_aug^T @ expT   (PE, ones row -> softmax denom)
    yT_h = y_augT[y rows] * (1/denom)    (DVE approx-reciprocal; denom
                                          broadcast via gpsimd
                                          partition_broadcast)
  out_partial = Y_local @ Wp_local       [T, 768]   (emitted one q-chunk
                                          late to avoid PE head-of-line
                                          blocking on the yT writes)

Host: out[b] = sum of the 4 partials + (b_proj + b_v @ W_proj).

Matmuls run in float32r (single-pass fp32, ~13 mantissa bits, ~2.2x
faster than the two-pass LOW_HIGH fp32 mode). Set MM_DT to
mybir.dt.float32 to go back to exact fp32.

qkT feature-chunk layout (matmul needs lhsT/rhs on the same base
partition, so each head's q and k live at the same partition offset):
  chunk0 = [q0 | q2], chunk1 = [k0 | k2], chunk2 = [q1], chunk3 = [k1]
yT layout [128, 2, T]: h0 -> (0:64, 0), h1 -> (64:128, 0), h2 -> (0:64, 1)
so the out-projection fuses h0+h1 into one K=128 matmul.
V_aug per-kb free layout [65 | 128 | 65]:
  h0: [v_h0, 1]; h1: [1, 0*63, v_h1] (y rows 64:128, denom row 0);
  h2: [v_h2, 1]
"""

import ml_dtypes
import numpy as np

import concourse.bass as bass
import concourse.mybir as mybir
import concourse.tile as tile
from concourse import bacc
from concourse import bass_utils

P = 128
D = 64          # head dim
HPC = 3         # heads per core
C = 768
CK = C // P     # 6 contraction chunks
QK = 2 * HPC * D  # 384 (q+k cols per core)
NH = 12
B = 2
N_CORES = 8
F32 = mybir.dt.float32
MM_DT = mybir.dt.bfloat16
NP_MM = ml_dtypes.bfloat16

# (partition offset, chunk idx) per head, for q and k
Q_POS = [(0, 0), (0, 2), (64, 0)]
K_POS = [(0, 1), (0, 3), (64, 1)]
# wqk DRAM column ranges per chunk: (start, width)
QK_CHUNKS = [(0, 128), (128, 128), (256, 64), (320, 64)]
# V_aug free-layout per head: (lhsT start, lhsT width, denom row, y row0)
V_SLICE = [(0, 65, 64, 0), (65, 128, 0, 64), (193, 65, 64, 0)]
VW = 258
# yT destination (row0, chunk) per head
Y_POS = [(0, 0), (64, 0), (0, 1)]


def build_nc(T=2048, QCW=512):
    """Build the per-core Bass program. T = sequence length, QCW = q-chunk."""
    assert T % QCW == 0 and QCW % P == 0 and T % 512 == 0
    NQC = T // QCW
    NTB = T // P
    NPH = C // 2  # 384, out-proj free-dim half

    nc = bacc.Bacc("TRN2", target_bir_lowering=False, debug=False,
                   num_devices=N_CORES)
    xT = nc.dram_tensor("xT", [C, T], MM_DT, kind="ExternalInput").ap()
    wqk = nc.dram_tensor("wqk", [C, QK], MM_DT, kind="ExternalInput").ap()
    wv = nc.dram_tensor("wv", [C, HPC * D], MM_DT, kind="ExternalInput").ap()
    bqk = nc.dram_tensor("bqk", [512], F32, kind="ExternalInput").ap()
    wp = nc.dram_tensor("wp", [2 * P, C], MM_DT, kind="ExternalInput").ap()
    out = nc.dram_tensor("out", [T, C], MM_DT, kind="ExternalOutput").ap()

    Exp = mybir.ActivationFunctionType.Exp

    with tile.TileContext(nc) as tc:
        with (
            tc.tile_pool(name="const", bufs=1) as const,
            tc.tile_pool(name="work", bufs=8) as work,
            tc.tile_pool(name="small", bufs=3) as small,
            tc.tile_pool(name="outp", bufs=3) as outp,
            tc.tile_pool(name="ps_mm", bufs=6, space="PSUM") as ps_mm,
            tc.tile_pool(name="ps_y", bufs=2, space="PSUM") as ps_y_pool,
        ):
            xT_sb = const.tile([P, CK, T], MM_DT, tag="xT")
            wqk_sb = const.tile([P, CK, QK], MM_DT, tag="wqk")
            wv_sb = const.tile([P, CK, HPC * D], MM_DT, tag="wv")
            bqk_sb = const.tile([P, 4], F32, tag="bqk")
            wp_sb = const.tile([P, 2, C], MM_DT, tag="wp")
            qkT_sb = const.tile([P, 4, T], MM_DT, tag="qkT")
            v_sb = const.tile([P, NTB, VW], MM_DT, tag="v")
            yT_sb = const.tile([P, 2, T], MM_DT, tag="yT")
            zb_sb = const.tile([P, 1], F32, tag="zb")
            mask_f = const.tile([P, 4, QCW], F32, tag="maskf")
            mask_sb = const.tile([P, 4, QCW], MM_DT, tag="mask")
            zq_sb = const.tile([P, 64], F32, tag="zq")

            # ---- constants first: masks on the (otherwise idle) DVE so
            # gpsimd's queue is free for DMA; small memsets on gpsimd ----
            st = const.tile([P, 2], F32, tag="st")
            st1 = const.tile([1, P], F32, tag="st1")
            ones_sb = const.tile([1, P], MM_DT, tag="ones")
            nc.gpsimd.memset(st[:], 1.0)
            nc.gpsimd.memset(st1[:], 1.0)
            nc.gpsimd.memset(zq_sb[:], 0.0)
            nc.gpsimd.memset(zb_sb[:], 0.0)
            # causal 0/1 masks: mask_j[x, y] = 1 if y - x >= 128*j else 0
            nc.vector.memset(mask_f[:], 1.0)
            for j in range(4):
                nc.gpsimd.affine_select(
                    mask_f[:, j, :], mask_f[:, j, :],
                    pattern=[[1, QCW]],
                    compare_op=mybir.AluOpType.is_ge,
                    fill=0.0,
                    base=-128 * j,
                    channel_multiplier=-1,
                )
            nc.vector.tensor_copy(mask_sb[:], mask_f[:])
            nc.vector.tensor_copy(ones_sb[:], st1[:])
            nc.vector.tensor_copy(v_sb[:, :, 64:66],
                                  st[:, None, :].to_broadcast((P, NTB, 2)))
            nc.vector.tensor_copy(v_sb[:, :, 257:258],
                                  st[:, None, 0:1].to_broadcast((P, NTB, 1)))
            # h1 junk cols (feed only never-read psy rows); zero for sim
            nc.vector.tensor_copy(v_sb[:, :, 66:129],
                                  zq_sb[:, None, 0:63].to_broadcast((P, NTB, 63)))

            # ---- loads. Queue-config costs ~650ns per dma_start on the
            # issuing sequencer, so the first qkT chain's inputs (wqk[kc],
            # xT[kc, 0:512]) alternate at the head of the sync and scalar
            # queues in kc order; everything else follows. The tj>=1 xT
            # tail is one transfer per kc. ----
            dma_engs = [nc.sync, nc.gpsimd, nc.scalar]
            for kc in range(CK):
                a, b = (nc.sync, nc.scalar) if kc % 2 == 0 else \
                    (nc.scalar, nc.sync)
                a.dma_start(wqk_sb[:, kc, :], wqk[kc * P:(kc + 1) * P, :])
                b.dma_start(xT_sb[:, kc, 0:512],
                            xT[kc * P:(kc + 1) * P, 0:512])
                if kc == 0:
                    nc.scalar.dma_start(
                        bqk_sb[:], bqk.rearrange("(ci p) -> p ci", p=P))
                if kc == 1:
                    nc.sync.dma_start(
                        wv_sb[:], wv.rearrange("(kc p) m -> p kc m", p=P))
            for kc in range(CK):
                [nc.gpsimd, nc.scalar, nc.sync][kc % 3].dma_start(
                    xT_sb[:, kc, 512:T], xT[kc * P:(kc + 1) * P, 512:T])
            nc.scalar.dma_start(wp_sb[:], wp.rearrange("(ci p) e -> p ci e", p=P))

            # ---- qkT + V projections for one 512-token slice.
            # chunks 2+3 ([q1|k1]) fused into one 128-row matmul chain; the
            # k1 half is relocated to chunk 3 partitions 0:64 by an
            # SBUF->SBUF DMA (engines can't cross partitions, DMA can). ----
            def project_tj(tj):
                tjs = slice(tj * 512, (tj + 1) * 512)
                for ci in range(2):
                    ps = ps_mm.tile([P, 512], F32, tag="mm")
                    for kc in range(CK):
                        nc.tensor.matmul(
                            ps[:],
                            wqk_sb[:, kc, ci * P:(ci + 1) * P],
                            xT_sb[:, kc, tjs],
                            start=(kc == 0), stop=(kc == CK - 1),
                        )
                    nc.vector.tensor_scalar_add(
                        qkT_sb[:, ci, tjs], ps[:], bqk_sb[:, ci:ci + 1])
                ps = ps_mm.tile([P, 512], F32, tag="mm")
                for kc in range(CK):
                    nc.tensor.matmul(
                        ps[:],
                        wqk_sb[:, kc, 256:384],
                        xT_sb[:, kc, tjs],
                        start=(kc == 0), stop=(kc == CK - 1),
                    )
                nc.vector.tensor_scalar_add(
                    qkT_sb[0:D, 2, tjs], ps[0:D, :], bqk_sb[0:D, 2:3])
                qkst = work.tile([P, 512], MM_DT, tag="qkst")
                nc.vector.tensor_copy(qkst[D:P, :], ps[D:P, :])
                nc.sync.dma_start(qkT_sb[0:D, 3, tjs], qkst[D:P, :])
                for tb in range(4 * tj, 4 * tj + 4):
                    ps = ps_mm.tile([P, 512], F32, tag="mm")
                    for kc in range(CK):
                        nc.tensor.matmul(
                            ps[:, :HPC * D],
                            xT_sb[:, kc, tb * P:(tb + 1) * P],
                            wv_sb[:, kc, :],
                            start=(kc == 0), stop=(kc == CK - 1),
                        )
                    nc.vector.tensor_copy(v_sb[:, tb, 0:64], ps[:, 0:64])
                    nc.vector.tensor_copy(v_sb[:, tb, 129:257],
                                          ps[:, 64:192])

            # ---- attention (per q-chunk) and delayed out-projection.
            # AV matmuls lag the scores matmuls by one kb step so the PE
            # never waits on the ACT exp chain; normalization of head i is
            # emitted during head i+1's matmul loop for the same reason.
            pend1, pend2 = [], []
            av_q = []

            AV_LAG = 1

            def flush_av(lag=AV_LAG):
                if len(av_q) >= lag:
                    av_q.pop(0)()

            def norm_stage1(st8):
                qc, h, psy_t = st8
                v0, vw, srow, yrow = V_SLICE[h]
                den = small.tile([1, QCW], F32, tag="den")
                nc.vector.tensor_copy(den[:], psy_t[srow:srow + 1, :])
                recf = small.tile([1, QCW], F32, tag="recf")
                nc.vector.reciprocal_approx_fast(recf[:], den[:])
                return (qc, h, psy_t, recf)

            def norm_stage2(st8):
                qc, h, psy_t, recf = st8
                q0 = qc * QCW
                v0, vw, srow, yrow = V_SLICE[h]
                bc = small.tile([P, QCW], F32, tag="bcs")
                nc.gpsimd.partition_broadcast(bc[:], recf[:])
                yp, yci = Y_POS[h]
                nc.vector.tensor_mul(
                    yT_sb[yp:yp + D, yci, q0:q0 + QCW],
                    psy_t[yrow:yrow + D, :], bc[yrow:yrow + D, :])

            def attn_qc(qc):
                q0 = qc * QCW
                kbmax = (q0 + QCW - 1) // P
                nfull = q0 // P  # blocks with no causal trim

                def mk_av(psy, kb, n0, v0, vw, exp_ap, last):
                    def av():
                        nc.tensor.matmul(
                            psy[:, n0:], v_sb[:, kb, v0:v0 + vw],
                            exp_ap[:, n0:],
                            start=(kb == 0), stop=last,
                        )
                    return av

                def norm_hook(step):
                    # one scores matmul was just issued; step counts them
                    # within the current head
                    if step == 2 and pend1:
                        pend2.append(norm_stage1(pend1.pop(0)))
                    elif step == 4 and pend2:
                        norm_stage2(pend2.pop(0))

                for h in range(HPC):
                    qp, qci = Q_POS[h]
                    kp, kci = K_POS[h]
                    v0, vw, srow, yrow = V_SLICE[h]
                    psy_t = ps_y_pool.tile([P, QCW], F32, tag="yaug",
                                           name="psy")
                    psy = psy_t[0:vw, :]
                    for kb in range(kbmax + 1):
                        # diagonal blocks only need q columns >= kb*128
                        n0 = max(0, kb * P - q0)
                        pss = ps_mm.tile([P, QCW], F32, tag="mm")
                        nc.tensor.matmul(
                            pss[:, n0:],
                            qkT_sb[kp:kp + D, kci, kb * P:(kb + 1) * P],
                            qkT_sb[qp:qp + D, qci, q0 + n0:q0 + QCW],
                            start=True, stop=True,
                        )
                        expT = work.tile([P, QCW], MM_DT, tag="expT")
                        nc.scalar.activation(expT[:, n0:], pss[:, n0:],
                                             Exp, bias=zb_sb[:])
                        if kb * P >= q0:  # diagonal block
                            nc.vector.tensor_mul(
                                expT[:, n0:], expT[:, n0:],
                                mask_sb[:, kb - q0 // P, n0:])
                        flush_av()
                        av_q.append(mk_av(psy, kb, n0, v0, vw, expT,
                                          kb == kbmax))
                        norm_hook(kb + 1)
                    pend1.append((qc, h, psy_t))

            def proj_qc(qc):
                q0 = qc * QCW
                for tb in range(q0 // P, (q0 + QCW) // P):
                    osb = outp.tile([P, C], MM_DT, tag="osb")
                    for half in range(2):
                        pso = ps_mm.tile([P, 512], F32, tag="mm",
                                         name="pso")[:, :NPH]
                        nc.tensor.matmul(
                            pso, yT_sb[:, 0, tb * P:(tb + 1) * P],
                            wp_sb[:, 0, half * NPH:(half + 1) * NPH],
                            start=True, stop=False)
                        nc.tensor.matmul(
                            pso, yT_sb[0:D, 1, tb * P:(tb + 1) * P],
                            wp_sb[0:D, 1, half * NPH:(half + 1) * NPH],
                            start=False, stop=True)
                        flush_av()
                        nc.vector.tensor_copy(
                            osb[:, half * NPH:(half + 1) * NPH], pso)
                    dma_engs[tb % 2].dma_start(
                        out[tb * P:(tb + 1) * P, :], osb[:])

            # interleaved pipeline: projections(tj) -> attention(tj) ->
            # out-projection(tj-1), so PE never queues behind a later
            # slice's xT DMA
            for tj in range(T // 512):
                project_tj(tj)
                attn_qc(tj)
                if tj > 0:
                    proj_qc(tj - 1)

            # ---- fast drain of the last q-chunk: the final head's norm
            # uses a PE broadcast (PE is idle here; the gpsimd broadcast
            # would sit on the critical path), and its yT multiply + out
            # projection are interleaved per 128-token block. The 4 final
            # stores fan out over 4 DMA queues. ----
            while av_q:
                av_q.pop(0)()
            qc = NQC - 1
            q0 = qc * QCW
            (qcl, hl, psy_l) = pend1.pop(0)
            assert (qcl, hl) == (qc, 2)
            v0, vw, srow, yrow = V_SLICE[hl]
            yp, yci = Y_POS[hl]
            den = small.tile([1, QCW], F32, tag="den")
            nc.vector.tensor_copy(den[:], psy_l[srow:srow + 1, :])
            recf = small.tile([1, QCW], F32, tag="recf")
            nc.vector.reciprocal_approx_fast(recf[:], den[:])
            recip_b = small.tile([1, QCW], MM_DT, tag="recb")
            nc.vector.tensor_copy(recip_b[:], recf[:])
            psb = ps_mm.tile([P, QCW], F32, tag="mm", name="psb")
            nc.tensor.matmul(psb[0:D, :], ones_sb[:, 0:D], recip_b[:],
                             start=True, stop=True)
            bcl = small.tile([P, QCW], F32, tag="bcs")
            store_engs = [nc.sync, nc.gpsimd, nc.scalar, nc.sync]
            for i, tb in enumerate(range(q0 // P, (q0 + QCW) // P)):
                c0 = i * P
                nc.scalar.copy(bcl[0:D, c0:c0 + P], psb[0:D, c0:c0 + P])
                nc.vector.tensor_mul(
                    yT_sb[yp:yp + D, yci, q0 + c0:q0 + c0 + P],
                    psy_l[yrow:yrow + D, c0:c0 + P], bcl[0:D, c0:c0 + P])
                osb = outp.tile([P, C], MM_DT, tag="osb")
                for half in range(2):
                    pso = ps_mm.tile([P, 512], F32, tag="mm",
                                     name="pso")[:, :NPH]
                    nc.tensor.matmul(
                        pso, yT_sb[:, 0, tb * P:(tb + 1) * P],
                        wp_sb[:, 0, half * NPH:(half + 1) * NPH],
                        start=True, stop=False)
                    nc.tensor.matmul(
                        pso, yT_sb[0:D, 1, tb * P:(tb + 1) * P],
                        wp_sb[0:D, 1, half * NPH:(half + 1) * NPH],
                        start=False, stop=True)
                    nc.vector.tensor_copy(
                        osb[:, half * NPH:(half + 1) * NPH], pso)
                store_engs[i].dma_start(
                    out[tb * P:(tb + 1) * P, :], osb[:])


    nc.compile()
    return nc


_NC_CACHE = {}


def _get_nc(T=2048, QCW=512):
    key = (T, QCW)
    if key not in _NC_CACHE:
        _NC_CACHE[key] = build_nc(T, QCW)
    return _NC_CACHE[key]


def build_in_maps(inputs):
    """Build the 8 per-core input dicts from full inputs."""
    x = np.asarray(inputs["x"], np.float32)
    W = np.asarray(inputs["W_attn"], np.float32)
    b = np.asarray(inputs["b_attn"], np.float32)
    W_proj = np.asarray(inputs["W_proj"], np.float32)
    in_maps = []
    for c in range(N_CORES):
        bi, g = divmod(c, 4)
        lo = g * (HPC * D)  # local head col offset within each of q/k/v
        qw = [W[:, lo + i * D:lo + (i + 1) * D] * 0.125 for i in range(HPC)]
        kw = [W[:, C + lo + i * D:C + lo + (i + 1) * D] for i in range(HPC)]
        qb = [b[lo + i * D:lo + (i + 1) * D] * 0.125 for i in range(HPC)]
        # chunk order: [q0|q2], [k0|k2], [q1], [k1]
        wqk = np.concatenate([qw[0], qw[2], kw[0], kw[2], qw[1], kw[1]],
                             axis=1)
        z64 = np.zeros(D, np.float32)
        bqk = np.concatenate([qb[0], qb[2], z64, z64, qb[1], z64, z64, z64])
        wv = W[:, 2 * C + lo:2 * C + lo + HPC * D]
        # wp rows: [h0 | h1 | h2 | zero pad] -> chunks (0:128), (128:256)
        wp = np.zeros((2 * P, C), np.float32)
        wp[:HPC * D] = W_proj[lo:lo + HPC * D]
        in_maps.append({
            "xT": np.ascontiguousarray(x[bi].T).astype(NP_MM),
            "wqk": np.ascontiguousarray(wqk).astype(NP_MM),
            "wv": np.ascontiguousarray(wv).astype(NP_MM),
            "bqk": np.ascontiguousarray(bqk),
            "wp": np.ascontiguousarray(wp).astype(NP_MM),
        })
    return in_maps


def postprocess(results, inputs):
    b_attn = np.asarray(inputs["b_attn"], np.float32)
    W_proj = np.asarray(inputs["W_proj"], np.float32)
    b_proj = np.asarray(inputs["b_proj"], np.float32)
    b_eff = b_proj + b_attn[2 * C:] @ W_proj
    T = results[0]["out"].shape[0]
    out = np.zeros((B, T, C), np.float32)
    for c in range(N_CORES):
        out[c // 4] += np.asarray(results[c]["out"], np.float32)
    out += b_eff
    return out


def kernel(x, W_attn, b_attn, W_proj, b_proj):
    inputs = dict(x=x, W_attn=W_attn, b_attn=b_attn,
                  W_proj=W_proj, b_proj=b_proj)
    T = np.asarray(x).shape[1]
    nc = _get_nc(T=T)
    in_maps = build_in_maps(inputs)
    res = bass_utils.run_bass_kernel_spmd(
        nc, in_maps, core_ids=list(range(N_CORES)))
    return postprocess(res.results, inputs)

